# revision 1
# baseline (speedup 1.0000x reference)
"""GRU cell kernel for Trainium2, data-parallel across 8 NeuronCores.

Per core: batch shard of 1024 rows; weights replicated.
  u  = sigmoid(x @ Wxu + h @ Whu + bu)
  r  = sigmoid(x @ Wxr + h @ Whr + br)
  c' = tanh  (x @ Wxc + (h*r) @ Whc + bc)
  c  = u*c' + (1-u)*h

Layout: all activations kept transposed in SBUF ([feature, batch]) so the
contraction dim lands on partitions; weights load in natural layout as the
stationary operand; matmuls run in float32r (full PE rate at 512-col moving).
x/h are transposed on entry and c back on exit via PE transposes.
"""

import os
import sys

import numpy as np

B = 8192
E = 1024
H = 1024
NCORES = 8
B_SH = B // NCORES  # 1024 rows per core

P = 128
KE = E // P   # 8 contraction chunks for x-side
KH = H // P   # 8 contraction chunks for h-side
NJ = H // P   # 8 output feature chunks
BN = 512      # moving free-dim per matmul (fp32 max)
NB = B_SH // BN  # 2

W_NAMES = ("Wxu", "Whu", "Wxr", "Whr", "Wxc", "Whc")
B_NAMES = ("bu", "br", "bc")

_NC_CACHE = {}


def _ensure_paths():
    for p in ("/opt/trn_rl_repo", "/root/.axon_site/_ro/trn_rl_repo"):
        if os.path.isdir(p) and p not in sys.path:
            sys.path.insert(0, p)


def _build_nc():
    import concourse.bass as bass
    import concourse.mybir as mybir
    from concourse.masks import make_identity
    from concourse.tile import TileContext

    f32 = mybir.dt.float32
    bf16 = mybir.dt.bfloat16
    AF = mybir.ActivationFunctionType

    nc = bass.Bass()
    x_d = nc.dram_tensor("input", [B_SH, E], f32, kind="ExternalInput")
    h_d = nc.dram_tensor("hidden_state", [B_SH, H], f32, kind="ExternalInput")
    w_d = {n: nc.dram_tensor(n, [E, H], f32, kind="ExternalInput") for n in W_NAMES}
    b_d = {n: nc.dram_tensor(n, [1, H], f32, kind="ExternalInput") for n in B_NAMES}
    out_d = nc.dram_tensor("output", [B_SH, H], f32, kind="ExternalOutput")

    with TileContext(nc) as tc:
        with (
            tc.tile_pool(name="sb", bufs=1) as sb,
            tc.tile_pool(name="psum", bufs=1, space="PSUM") as pp,
        ):
            ident = sb.tile([P, P], f32, tag="ident", bufs=1)
            make_identity(nc, ident[:])

            xT = [sb.tile([P, B_SH], bf16, tag=f"xT{k}", name=f"xT{k}", bufs=1) for k in range(KE)]
            hT = [sb.tile([P, B_SH], bf16, tag=f"hT{k}", name=f"hT{k}", bufs=1) for k in range(KH)]
            uT = [sb.tile([P, B_SH], f32, tag=f"uT{j}", name=f"uT{j}", bufs=1) for j in range(NJ)]
            rhT = [sb.tile([P, B_SH], bf16, tag=f"rhT{j}", name=f"rhT{j}", bufs=1) for j in range(NJ)]
            hT32 = [sb.tile([P, B_SH], f32, tag=f"hT32{k}", name=f"hT32{k}", bufs=1) for k in range(KH)]

            # ---- load x, h and transpose into [feature, batch] layout ----
            for src_d, dstT in ((x_d, xT), (h_d, hT)):
                for bi in range(B_SH // P):
                    nat = sb.tile([P, E], f32, tag="nat", bufs=3)
                    nc.sync.dma_start(nat[:], src_d[bi * P : (bi + 1) * P, :])
                    for ej in range(KE):
                        ps = pp.tile([P, P], f32, tag="pstr", bufs=4)
                        nc.tensor.transpose(
                            ps[:], nat[:, ej * P : (ej + 1) * P], ident[:]
                        )
                        nc.vector.tensor_copy(
                            dstT[ej][:, bi * P : (bi + 1) * P], ps[:]
                        )
                        if dstT is hT:
                            nc.vector.tensor_copy(
                                hT32[ej][:, bi * P : (bi + 1) * P], ps[:]
                            )

            bias_t = {}
            for g, nm in (("u", "bu"), ("r", "br"), ("c", "bc")):
                bt = sb.tile([P, NJ], f32, tag=f"bias_{g}", bufs=1)
                for j in range(NJ):
                    nc.sync.dma_start(
                        bt[:, j : j + 1],
                        b_d[nm][0:1, j * P : (j + 1) * P].rearrange("a p -> p a"),
                    )
                bias_t[g] = bt

            def dma_w_ktiles(wname):
                tiles = []
                for k in range(KE):
                    ws = sb.tile([P, E], f32, tag="wstage", name=f"ws_{wname}_{k}", bufs=3)
                    nc.sync.dma_start(ws[:], w_d[wname][k * P : (k + 1) * P, :])
                    wt = sb.tile([P, E], bf16, tag="w", name=f"w_{wname}_{k}", bufs=18)
                    nc.vector.tensor_copy(wt[:], ws[:])
                    tiles.append(wt)
                return tiles

            def gate_matmuls(ps, wxs, whs, rhsT, j, n):
                jsl = slice(j * P, (j + 1) * P)
                sl = slice(n * BN, (n + 1) * BN)
                for k in range(KE):
                    nc.tensor.matmul(
                        ps[:],
                        wxs[k][:, jsl],
                        xT[k][:, sl],
                        start=(k == 0),
                        stop=False,
                    )
                for k in range(KH):
                    nc.tensor.matmul(
                        ps[:],
                        whs[k][:, jsl],
                        rhsT[k][:, sl],
                        start=False,
                        stop=(k == KH - 1),
                    )

            # ---- gate r, then u (both sigmoid); r is folded into r*h ----
            for gname, wx, wh, dst in (("r", "Wxr", "Whr", rhT), ("u", "Wxu", "Whu", uT)):
                wxs = dma_w_ktiles(wx)
                whs = dma_w_ktiles(wh)
                for j in range(NJ):
                    for n in range(NB):
                        sl = slice(n * BN, (n + 1) * BN)
                        ps = pp.tile([P, BN], f32, tag="mm", bufs=4)
                        gate_matmuls(ps, wxs, whs, hT, j, n)
                        nc.scalar.activation(
                            dst[j][:, sl], ps[:], AF.Sigmoid,
                            bias=bias_t[gname][:, j : j + 1],
                        )
                        if gname == "r":
                            nc.vector.tensor_mul(
                                dst[j][:, sl], dst[j][:, sl], hT[j][:, sl]
                            )

            # ---- candidate + blend + output transpose, per feature chunk ----
            wxs = dma_w_ktiles("Wxc")
            whs = dma_w_ktiles("Whc")
            for j in range(NJ):
                cc = sb.tile([P, B_SH], f32, tag="cc", bufs=3)
                for n in range(NB):
                    sl = slice(n * BN, (n + 1) * BN)
                    ps = pp.tile([P, BN], f32, tag="mm", bufs=4)
                    gate_matmuls(ps, wxs, whs, rhT, j, n)
                    nc.scalar.activation(
                        cc[:, sl], ps[:], AF.Tanh, bias=bias_t["c"][:, j : j + 1]
                    )
                    # c = h + u*(c' - h), computed in place in cc
                    nc.vector.tensor_sub(cc[:, sl], cc[:, sl], hT32[j][:, sl])
                    nc.vector.tensor_mul(cc[:, sl], cc[:, sl], uT[j][:, sl])
                    nc.vector.tensor_add(cc[:, sl], cc[:, sl], hT32[j][:, sl])
                for bi in range(B_SH // P):
                    ps = pp.tile([P, P], f32, tag="pstr", bufs=4)
                    nc.tensor.transpose(
                        ps[:], cc[:, bi * P : (bi + 1) * P], ident[:]
                    )
                    ot = sb.tile([P, P], f32, tag="ost", bufs=4)
                    nc.vector.tensor_copy(ot[:], ps[:])
                    nc.sync.dma_start(
                        out_d[bi * P : (bi + 1) * P, j * P : (j + 1) * P], ot[:]
                    )

    _split_matmul_waits(nc, mybir)
    return nc


def _split_matmul_waits(nc, mybir):
    """Walrus codegen allows only one sync-wait on a Matmult (it lowers to an
    LDW+MM pair).  Spill extra waits onto a PE NoOp placed just before."""
    n_fixed = 0
    blocks = list(nc.m.functions[0].blocks)
    origs = [list(b.instructions) for b in blocks]
    spill_nops = {}  # id(inst) -> [nop insts]
    for orig in origs:
        for inst in orig:
            si = inst.sync_info
            if (
                si is not None
                and si.on_wait
                and len(si.on_wait) > 1
            ):
                waits = list(si.on_wait)
                eng = nc.engines[inst.engine]
                nops = []
                for w in waits[:-1]:
                    nop = eng.nop(hint="waitspill").ins
                    nop.sync_info = mybir.SyncInfo(on_wait=[w], on_update=[])
                    nops.append(nop)
                inst.sync_info = mybir.SyncInfo(
                    on_wait=waits[-1:], on_update=list(si.on_update or [])
                )
                spill_nops[id(inst)] = nops
                n_fixed += 1
    for blk, orig in zip(blocks, origs):
        new_list = []
        for inst in orig:
            if id(inst) in spill_nops:
                new_list.extend(spill_nops[id(inst)])
            new_list.append(inst)
        # rebuilding from `orig` also drops any freshly created nops that
        # bass appended to this block's tail
        blk.instructions[:] = new_list
    return n_fixed


def get_nc():
    if "nc" not in _NC_CACHE:
        _ensure_paths()
        _NC_CACHE["nc"] = _build_nc()
    return _NC_CACHE["nc"]


def kernel(**inputs):
    _ensure_paths()
    from concourse.bass_utils import run_bass_kernel_spmd

    nc = get_nc()

    x = np.ascontiguousarray(np.asarray(inputs["input"], dtype=np.float32))
    h = np.ascontiguousarray(np.asarray(inputs["hidden_state"], dtype=np.float32))
    shared = {
        n: np.ascontiguousarray(np.asarray(inputs[n], dtype=np.float32))
        for n in W_NAMES + B_NAMES
    }
    in_maps = []
    for c in range(NCORES):
        m = {
            "input": x[c * B_SH : (c + 1) * B_SH],
            "hidden_state": h[c * B_SH : (c + 1) * B_SH],
        }
        m.update(shared)
        in_maps.append(m)

    res = run_bass_kernel_spmd(nc, in_maps, list(range(NCORES)))
    out = np.concatenate(
        [np.asarray(res.results[c]["output"]) for c in range(NCORES)], axis=0
    )
    return out.astype(np.float32)



# revision 3
# speedup vs baseline: 1.4767x; 1.4767x over previous
"""GRU cell kernel for Trainium2, data-parallel across 8 NeuronCores.

Per core: batch shard of 1024 rows; weights replicated.
  u  = sigmoid(x @ Wxu + h @ Whu + bu)
  r  = sigmoid(x @ Wxr + h @ Whr + br)
  c' = tanh  (x @ Wxc + (h*r) @ Whc + bc)
  c  = u*c' + (1-u)*h

All layout work happens on the host (free - the harness times only the
NEFF): x and h are transposed to [feature, batch] and converted to bf16
before upload, weights are uploaded as bf16 in natural layout, and the
output comes back transposed [H, batch] fp32 and is untransposed on the
host.  On-chip the kernel is a pure matmul stream: 768 bf16 matmuls
(stationary = 128-col weight slice, moving = 512-col activation slice)
plus scalar-engine activations and a vector-engine blend.
"""

import os
import sys

import numpy as np

B = 8192
E = 1024
H = 1024
NCORES = 8
B_SH = B // NCORES  # 1024 rows per core

P = 128
KE = E // P   # 8 contraction chunks
NJ = H // P   # 8 output feature chunks
BN = 512      # moving free-dim per matmul (one PSUM bank of fp32)
NB = B_SH // BN  # 2

W_NAMES = ("Wxu", "Whu", "Wxr", "Whr", "Wxc", "Whc")
B_NAMES = ("bu", "br", "bc")

_NC_CACHE = {}


def _ensure_paths():
    for p in ("/opt/trn_rl_repo", "/root/.axon_site/_ro/trn_rl_repo"):
        if os.path.isdir(p) and p not in sys.path:
            sys.path.insert(0, p)


def _build_nc():
    import concourse.bass as bass
    import concourse.mybir as mybir
    from concourse.tile import TileContext

    f32 = mybir.dt.float32
    bf16 = mybir.dt.bfloat16
    AF = mybir.ActivationFunctionType

    nc = bass.Bass()
    x_d = nc.dram_tensor("xT", [E, B_SH], bf16, kind="ExternalInput")
    h_d = nc.dram_tensor("hT", [H, B_SH], bf16, kind="ExternalInput")
    w_d = {n: nc.dram_tensor(n, [E, H], bf16, kind="ExternalInput") for n in W_NAMES}
    b_d = nc.dram_tensor("btab", [P, 3 * NJ], f32, kind="ExternalInput")
    out_d = nc.dram_tensor("out", [H, B_SH], f32, kind="ExternalOutput")

    with TileContext(nc) as tc:
        with (
            tc.tile_pool(name="sb", bufs=1) as sb,
            tc.tile_pool(name="psum", bufs=1, space="PSUM") as pp,
        ):
            xt = [sb.tile([P, B_SH], bf16, tag=f"xt{k}", name=f"xt{k}", bufs=1) for k in range(KE)]
            ht = [sb.tile([P, B_SH], bf16, tag=f"ht{k}", name=f"ht{k}", bufs=1) for k in range(KE)]
            wt = {
                n: [sb.tile([P, H], bf16, tag=f"w_{n}_{k}", name=f"w_{n}_{k}", bufs=1) for k in range(KE)]
                for n in W_NAMES
            }
            ut = [sb.tile([P, B_SH], f32, tag=f"ut{j}", name=f"ut{j}", bufs=1) for j in range(NJ)]
            rh = [sb.tile([P, B_SH], bf16, tag=f"rh{j}", name=f"rh{j}", bufs=1) for j in range(NJ)]
            bias = sb.tile([P, 3 * NJ], f32, tag="bias", bufs=1)

            # ---- DMAs in consumption order (sync ring is FIFO) ----
            for k in range(KE):
                nc.sync.dma_start(xt[k][:], x_d[k * P : (k + 1) * P, :])
                nc.sync.dma_start(wt["Wxr"][k][:], w_d["Wxr"][k * P : (k + 1) * P, :])
            for k in range(KE):
                nc.sync.dma_start(ht[k][:], h_d[k * P : (k + 1) * P, :])
                nc.sync.dma_start(wt["Whr"][k][:], w_d["Whr"][k * P : (k + 1) * P, :])
            nc.sync.dma_start(bias[:], b_d[:, :])
            for k in range(KE):
                nc.sync.dma_start(wt["Wxu"][k][:], w_d["Wxu"][k * P : (k + 1) * P, :])
                nc.sync.dma_start(wt["Whu"][k][:], w_d["Whu"][k * P : (k + 1) * P, :])
            for k in range(KE):
                nc.sync.dma_start(wt["Wxc"][k][:], w_d["Wxc"][k * P : (k + 1) * P, :])
                nc.sync.dma_start(wt["Whc"][k][:], w_d["Whc"][k * P : (k + 1) * P, :])

            def gate_psums(wx, wh, hside, j):
                """16-matmul accumulation chains for output chunk j, both
                batch halves interleaved so consecutive matmuls share the
                stationary operand."""
                jsl = slice(j * P, (j + 1) * P)
                ps = [pp.tile([P, BN], f32, tag="mm", name=f"ps{j}_{_n}", bufs=8) for _n in range(NB)]
                for k in range(KE):
                    for n in range(NB):
                        nc.tensor.matmul(
                            ps[n][:],
                            wt[wx][k][:, jsl],
                            xt[k][:, n * BN : (n + 1) * BN],
                            start=(k == 0),
                            stop=False,
                        )
                for k in range(KE):
                    for n in range(NB):
                        nc.tensor.matmul(
                            ps[n][:],
                            wt[wh][k][:, jsl],
                            hside[k][:, n * BN : (n + 1) * BN],
                            start=False,
                            stop=(k == KE - 1),
                        )
                return ps

            # ---- gate r: sigmoid -> multiply by h (kept transposed) ----
            for j in range(NJ):
                ps = gate_psums("Wxr", "Whr", ht, j)
                for n in range(NB):
                    sl = slice(n * BN, (n + 1) * BN)
                    nc.scalar.activation(
                        rh[j][:, sl], ps[n][:], AF.Sigmoid, bias=bias[:, j : j + 1]
                    )
                nc.vector.tensor_mul(rh[j][:], rh[j][:], ht[j][:])

            # ---- gate u: sigmoid, kept fp32 ----
            for j in range(NJ):
                ps = gate_psums("Wxu", "Whu", ht, j)
                for n in range(NB):
                    sl = slice(n * BN, (n + 1) * BN)
                    nc.scalar.activation(
                        ut[j][:, sl], ps[n][:], AF.Sigmoid,
                        bias=bias[:, NJ + j : NJ + j + 1],
                    )

            # ---- candidate + blend + store ----
            for j in range(NJ):
                ps = gate_psums("Wxc", "Whc", rh, j)
                cc = sb.tile([P, B_SH], f32, tag="cc", bufs=4)
                for n in range(NB):
                    sl = slice(n * BN, (n + 1) * BN)
                    nc.scalar.activation(
                        cc[:, sl], ps[n][:], AF.Tanh,
                        bias=bias[:, 2 * NJ + j : 2 * NJ + j + 1],
                    )
                # c = h + u*(c' - h)
                nc.vector.tensor_sub(cc[:], cc[:], ht[j][:])
                nc.vector.tensor_mul(cc[:], cc[:], ut[j][:])
                nc.vector.tensor_add(cc[:], cc[:], ht[j][:])
                nc.sync.dma_start(out_d[j * P : (j + 1) * P, :], cc[:])

    _split_matmul_waits(nc, mybir)
    return nc


def _split_matmul_waits(nc, mybir):
    """Walrus codegen allows only one sync-wait on a Matmult (it lowers to an
    LDW+MM pair).  Spill extra waits onto a PE NoOp placed just before."""
    n_fixed = 0
    blocks = list(nc.m.functions[0].blocks)
    origs = [list(b.instructions) for b in blocks]
    spill_nops = {}  # id(inst) -> [nop insts]
    for orig in origs:
        for inst in orig:
            si = inst.sync_info
            if (
                si is not None
                and si.on_wait
                and len(si.on_wait) > 1
            ):
                waits = list(si.on_wait)
                eng = nc.engines[inst.engine]
                nops = []
                for w in waits[:-1]:
                    nop = eng.nop(hint="waitspill").ins
                    nop.sync_info = mybir.SyncInfo(on_wait=[w], on_update=[])
                    nops.append(nop)
                inst.sync_info = mybir.SyncInfo(
                    on_wait=waits[-1:], on_update=list(si.on_update or [])
                )
                spill_nops[id(inst)] = nops
                n_fixed += 1
    for blk, orig in zip(blocks, origs):
        new_list = []
        for inst in orig:
            if id(inst) in spill_nops:
                new_list.extend(spill_nops[id(inst)])
            new_list.append(inst)
        # rebuilding from `orig` also drops any freshly created nops that
        # bass appended to this block's tail
        blk.instructions[:] = new_list
    return n_fixed


def get_nc():
    if "nc" not in _NC_CACHE:
        _ensure_paths()
        _NC_CACHE["nc"] = _build_nc()
    return _NC_CACHE["nc"]


def build_in_maps(inputs):
    """Host-side prep: transpose x/h, convert to bf16, pack biases."""
    import ml_dtypes

    bf = ml_dtypes.bfloat16
    x = np.asarray(inputs["input"], dtype=np.float32)
    h = np.asarray(inputs["hidden_state"], dtype=np.float32)
    xT = np.ascontiguousarray(x.astype(bf).T)  # [E, B]
    hT = np.ascontiguousarray(h.astype(bf).T)  # [H, B]
    shared = {
        n: np.ascontiguousarray(np.asarray(inputs[n], dtype=np.float32).astype(bf))
        for n in W_NAMES
    }
    btab = np.zeros((P, 3 * NJ), np.float32)
    for g, nm in enumerate(("br", "bu", "bc")):
        b = np.asarray(inputs[nm], dtype=np.float32).reshape(H)
        btab[:, g * NJ : (g + 1) * NJ] = b.reshape(NJ, P).T
    shared["btab"] = btab

    in_maps = []
    for c in range(NCORES):
        m = {
            "xT": np.ascontiguousarray(xT[:, c * B_SH : (c + 1) * B_SH]),
            "hT": np.ascontiguousarray(hT[:, c * B_SH : (c + 1) * B_SH]),
        }
        m.update(shared)
        in_maps.append(m)
    return in_maps


def assemble_output(res):
    outT = np.concatenate(
        [np.asarray(res.results[c]["out"]) for c in range(NCORES)], axis=1
    )  # [H, B]
    return np.ascontiguousarray(outT.T).astype(np.float32)


def kernel(**inputs):
    _ensure_paths()
    from concourse.bass_utils import run_bass_kernel_spmd

    nc = get_nc()
    in_maps = build_in_maps(inputs)
    res = run_bass_kernel_spmd(nc, in_maps, list(range(NCORES)))
    return assemble_output(res)


# revision 5
# speedup vs baseline: 1.4883x; 1.0079x over previous
"""GRU cell kernel for Trainium2, data-parallel across 8 NeuronCores.

Per core: batch shard of 1024 rows; weights replicated.
  u  = sigmoid(x @ Wxu + h @ Whu + bu)
  r  = sigmoid(x @ Wxr + h @ Whr + br)
  c' = tanh  (x @ Wxc + (h*r) @ Whc + bc)
  c  = u*c' + (1-u)*h

All layout work happens on the host (free - the harness times only the
NEFF): x and h are transposed to [feature, batch] and converted to bf16
before upload, weights are uploaded as bf16 in natural layout, and the
output comes back transposed [H, batch] fp32 and is untransposed on the
host.  On-chip the kernel is a pure matmul stream: 768 bf16 matmuls
(stationary = 128-col weight slice, moving = 512-col activation slice)
plus scalar-engine activations and a vector-engine blend.
"""

import os
import sys

import numpy as np

B = 8192
E = 1024
H = 1024
NCORES = 8
B_SH = B // NCORES  # 1024 rows per core

P = 128
KE = E // P   # 8 contraction chunks
NJ = H // P   # 8 output feature chunks
BN = 512      # moving free-dim per matmul (one PSUM bank of fp32)
NB = B_SH // BN  # 2

W_NAMES = ("Wxu", "Whu", "Wxr", "Whr", "Wxc", "Whc")
B_NAMES = ("bu", "br", "bc")

_NC_CACHE = {}


def _ensure_paths():
    for p in ("/opt/trn_rl_repo", "/root/.axon_site/_ro/trn_rl_repo"):
        if os.path.isdir(p) and p not in sys.path:
            sys.path.insert(0, p)


def _build_nc():
    import concourse.bass as bass
    import concourse.mybir as mybir
    from concourse.tile import TileContext

    f32 = mybir.dt.float32
    bf16 = mybir.dt.bfloat16
    AF = mybir.ActivationFunctionType

    nc = bass.Bass()
    x_d = nc.dram_tensor("xT", [E, B_SH], bf16, kind="ExternalInput")
    h_d = nc.dram_tensor("hT", [H, B_SH], bf16, kind="ExternalInput")
    w_d = {n: nc.dram_tensor(n, [E, H], bf16, kind="ExternalInput") for n in W_NAMES}
    b_d = nc.dram_tensor("btab", [P, 3 * NJ], f32, kind="ExternalInput")
    out_d = nc.dram_tensor("out", [H, B_SH], f32, kind="ExternalOutput")

    with TileContext(nc) as tc:
        with (
            tc.tile_pool(name="sb", bufs=1) as sb,
            tc.tile_pool(name="psum", bufs=1, space="PSUM") as pp,
        ):
            xt = [sb.tile([P, B_SH], bf16, tag=f"xt{k}", name=f"xt{k}", bufs=1) for k in range(KE)]
            ht = [sb.tile([P, B_SH], bf16, tag=f"ht{k}", name=f"ht{k}", bufs=1) for k in range(KE)]
            wt = {
                n: [sb.tile([P, H], bf16, tag=f"w_{n}_{k}", name=f"w_{n}_{k}", bufs=1) for k in range(KE)]
                for n in W_NAMES
            }
            ut = [sb.tile([P, B_SH], f32, tag=f"ut{j}", name=f"ut{j}", bufs=1) for j in range(NJ)]
            rh = [sb.tile([P, B_SH], bf16, tag=f"rh{j}", name=f"rh{j}", bufs=1) for j in range(NJ)]
            # uh1m = (u-1)*h, precomputed off the critical path: the final
            # blend is then c = c'*u - uh1m (two vector ops at the tail).
            uh1m = [sb.tile([P, B_SH], bf16, tag=f"uh{j}", name=f"uh{j}", bufs=1) for j in range(NJ)]
            bias = sb.tile([P, 3 * NJ], f32, tag="bias", bufs=1)

            # ---- DMAs in consumption order (sync ring is FIFO) ----
            for k in range(KE):
                nc.sync.dma_start(xt[k][:], x_d[k * P : (k + 1) * P, :])
                nc.sync.dma_start(wt["Wxr"][k][:], w_d["Wxr"][k * P : (k + 1) * P, :])
            for k in range(KE):
                nc.sync.dma_start(ht[k][:], h_d[k * P : (k + 1) * P, :])
                nc.sync.dma_start(wt["Whr"][k][:], w_d["Whr"][k * P : (k + 1) * P, :])
            nc.sync.dma_start(bias[:], b_d[:, :])
            for k in range(KE):
                nc.sync.dma_start(wt["Wxu"][k][:], w_d["Wxu"][k * P : (k + 1) * P, :])
                nc.sync.dma_start(wt["Whu"][k][:], w_d["Whu"][k * P : (k + 1) * P, :])
            for k in range(KE):
                nc.sync.dma_start(wt["Wxc"][k][:], w_d["Wxc"][k * P : (k + 1) * P, :])
                nc.sync.dma_start(wt["Whc"][k][:], w_d["Whc"][k * P : (k + 1) * P, :])

            def gate_psums(wx, wh, hside, j):
                """16-matmul accumulation chains for output chunk j, both
                batch halves interleaved so consecutive matmuls share the
                stationary operand."""
                jsl = slice(j * P, (j + 1) * P)
                ps = [pp.tile([P, BN], f32, tag="mm", name=f"ps{j}_{_n}", bufs=8) for _n in range(NB)]
                for k in range(KE):
                    for n in range(NB):
                        nc.tensor.matmul(
                            ps[n][:],
                            wt[wx][k][:, jsl],
                            xt[k][:, n * BN : (n + 1) * BN],
                            start=(k == 0),
                            stop=False,
                        )
                for k in range(KE):
                    for n in range(NB):
                        nc.tensor.matmul(
                            ps[n][:],
                            wt[wh][k][:, jsl],
                            hside[k][:, n * BN : (n + 1) * BN],
                            start=False,
                            stop=(k == KE - 1),
                        )
                return ps

            # ---- gate r: sigmoid -> multiply by h (kept transposed) ----
            # k-outer over groups of 4 j's (8 PSUM banks) so the matmul
            # stream keeps pace with the weight/activation DMAs still in
            # flight at kernel start instead of stalling inside one chain.
            for jlo in range(0, NJ, 4):
                grp = range(jlo, jlo + 4)
                gps = {
                    j: [
                        pp.tile([P, BN], f32, tag="mm", name=f"psr{j}_{_n}", bufs=8)
                        for _n in range(NB)
                    ]
                    for j in grp
                }
                for k in range(KE):
                    for j in grp:
                        for n in range(NB):
                            nc.tensor.matmul(
                                gps[j][n][:],
                                wt["Wxr"][k][:, j * P : (j + 1) * P],
                                xt[k][:, n * BN : (n + 1) * BN],
                                start=(k == 0),
                                stop=False,
                            )
                for k in range(KE):
                    for j in grp:
                        for n in range(NB):
                            nc.tensor.matmul(
                                gps[j][n][:],
                                wt["Whr"][k][:, j * P : (j + 1) * P],
                                ht[k][:, n * BN : (n + 1) * BN],
                                start=False,
                                stop=(k == KE - 1),
                            )
                for j in grp:
                    for n in range(NB):
                        sl = slice(n * BN, (n + 1) * BN)
                        nc.scalar.activation(
                            rh[j][:, sl], gps[j][n][:], AF.Sigmoid,
                            bias=bias[:, j : j + 1],
                        )
                    nc.vector.tensor_mul(rh[j][:], rh[j][:], ht[j][:])

            # ---- gate u: sigmoid fp32; also precompute (u-1)*h ----
            for j in range(NJ):
                ps = gate_psums("Wxu", "Whu", ht, j)
                for n in range(NB):
                    sl = slice(n * BN, (n + 1) * BN)
                    nc.scalar.activation(
                        ut[j][:, sl], ps[n][:], AF.Sigmoid,
                        bias=bias[:, NJ + j : NJ + j + 1],
                    )
                nc.vector.scalar_tensor_tensor(
                    uh1m[j][:], ut[j][:], 1.0, ht[j][:],
                    op0=mybir.AluOpType.subtract, op1=mybir.AluOpType.mult,
                )

            # ---- candidate + blend + store, pipelined per batch half ----
            for j in range(NJ):
                ps = gate_psums("Wxc", "Whc", rh, j)
                cc = sb.tile([P, B_SH], f32, tag="cc", bufs=3)
                for n in range(NB):
                    sl = slice(n * BN, (n + 1) * BN)
                    nc.scalar.activation(
                        cc[:, sl], ps[n][:], AF.Tanh,
                        bias=bias[:, 2 * NJ + j : 2 * NJ + j + 1],
                    )
                    # c = c'*u - (u-1)*h
                    nc.vector.tensor_mul(cc[:, sl], cc[:, sl], ut[j][:, sl])
                    nc.vector.tensor_sub(cc[:, sl], cc[:, sl], uh1m[j][:, sl])
                    nc.sync.dma_start(
                        out_d[j * P : (j + 1) * P, n * BN : (n + 1) * BN],
                        cc[:, sl],
                    )

    _split_matmul_waits(nc, mybir)
    return nc


def _split_matmul_waits(nc, mybir):
    """Walrus codegen allows only one sync-wait on a Matmult (it lowers to an
    LDW+MM pair).  Spill extra waits onto a PE NoOp placed just before."""
    n_fixed = 0
    blocks = list(nc.m.functions[0].blocks)
    origs = [list(b.instructions) for b in blocks]
    spill_nops = {}  # id(inst) -> [nop insts]
    for orig in origs:
        for inst in orig:
            si = inst.sync_info
            if (
                si is not None
                and si.on_wait
                and len(si.on_wait) > 1
            ):
                waits = list(si.on_wait)
                eng = nc.engines[inst.engine]
                nops = []
                for w in waits[:-1]:
                    nop = eng.nop(hint="waitspill").ins
                    nop.sync_info = mybir.SyncInfo(on_wait=[w], on_update=[])
                    nops.append(nop)
                inst.sync_info = mybir.SyncInfo(
                    on_wait=waits[-1:], on_update=list(si.on_update or [])
                )
                spill_nops[id(inst)] = nops
                n_fixed += 1
    for blk, orig in zip(blocks, origs):
        new_list = []
        for inst in orig:
            if id(inst) in spill_nops:
                new_list.extend(spill_nops[id(inst)])
            new_list.append(inst)
        # rebuilding from `orig` also drops any freshly created nops that
        # bass appended to this block's tail
        blk.instructions[:] = new_list
    return n_fixed


def get_nc():
    if "nc" not in _NC_CACHE:
        _ensure_paths()
        _NC_CACHE["nc"] = _build_nc()
    return _NC_CACHE["nc"]


def build_in_maps(inputs):
    """Host-side prep: transpose x/h, convert to bf16, pack biases."""
    import ml_dtypes

    bf = ml_dtypes.bfloat16
    x = np.asarray(inputs["input"], dtype=np.float32)
    h = np.asarray(inputs["hidden_state"], dtype=np.float32)
    xT = np.ascontiguousarray(x.astype(bf).T)  # [E, B]
    hT = np.ascontiguousarray(h.astype(bf).T)  # [H, B]
    shared = {
        n: np.ascontiguousarray(np.asarray(inputs[n], dtype=np.float32).astype(bf))
        for n in W_NAMES
    }
    btab = np.zeros((P, 3 * NJ), np.float32)
    for g, nm in enumerate(("br", "bu", "bc")):
        b = np.asarray(inputs[nm], dtype=np.float32).reshape(H)
        btab[:, g * NJ : (g + 1) * NJ] = b.reshape(NJ, P).T
    shared["btab"] = btab

    in_maps = []
    for c in range(NCORES):
        m = {
            "xT": np.ascontiguousarray(xT[:, c * B_SH : (c + 1) * B_SH]),
            "hT": np.ascontiguousarray(hT[:, c * B_SH : (c + 1) * B_SH]),
        }
        m.update(shared)
        in_maps.append(m)
    return in_maps


def assemble_output(res):
    outT = np.concatenate(
        [np.asarray(res.results[c]["out"]) for c in range(NCORES)], axis=1
    )  # [H, B]
    return np.ascontiguousarray(outT.T).astype(np.float32)


def kernel(**inputs):
    _ensure_paths()
    from concourse.bass_utils import run_bass_kernel_spmd

    nc = get_nc()
    in_maps = build_in_maps(inputs)
    res = run_bass_kernel_spmd(nc, in_maps, list(range(NCORES)))
    return assemble_output(res)
